# revision 39
# baseline (speedup 1.0000x reference)
"""Trainium2 Bass kernel for nn_DXVAE (GNN message-passing encoder).

Contract: kernel(**inputs) takes the FULL (unsharded) numpy inputs of
reference.setup_inputs() and returns the full (mu, std) outputs.

Strategy: pure data parallelism over the batch (2048 graphs -> 256 per
NeuronCore x 8 cores), weights replicated. Inside each core the per-node
sequential loop (v = 6..0) runs with activations in transposed layout
[H on partitions, batch on free dim].

Key algebraic restructure vs the reference: the reference computes, for
every node v, gate/mapper matmuls over the stacked masked neighbors
(Hcat [B,k,2H] @ [2H,H]).  Because the masks are per-(batch,pair) scalars,
   sigmoid(Hcat @ Wg.T + bg) = sigmoid(p*A_x + s*B_x + bg)
with A_x = Wg[:, :H] @ h_x, B_x = Wg[:, H:] @ h_x (and C_x, D_x for Wm).
So we compute the four projections ONCE per node x (4H^2 MACs) and each
(v, x) pair becomes cheap elementwise work accumulated eagerly into
per-v accumulators.  This cuts the dominant matmul FLOPs ~3.5x.
"""

import sys
import numpy as np

sys.path.insert(0, "/opt/trn_rl_repo")

import ml_dtypes
from contextlib import ExitStack

import concourse.bass as bass
import concourse.mybir as mybir
import concourse.tile as tile
from concourse import bacc
from concourse.bass_utils import run_bass_kernel_spmd
from concourse.masks import make_identity

F32 = mybir.dt.float32
BF16 = mybir.dt.bfloat16
AF = mybir.ActivationFunctionType
ALU = mybir.AluOpType

N_CORES = 8
B = 2048
BL = B // N_CORES          # 256 batch per core
NN = 7                     # nodes
H = 1024
HT = H // 128              # 8 H chunks
Z = 256
XD = 27
X0D = 23

N_PAIRS = (NN - 1) * NN // 2 - 0  # pairs (v, x) with 0 <= v < x <= 6, v<=5 -> 21


def _pair_id(v, x):
    # enumerate pairs (v, x), v < x, ordered
    pid = 0
    for xx in range(1, NN):
        for vv in range(xx):
            if vv == v and xx == x:
                return pid
            pid += 1
    raise ValueError((v, x))


def _bf(a):
    return np.ascontiguousarray(np.asarray(a, dtype=np.float32)).astype(ml_dtypes.bfloat16)


def _f32(a):
    return np.ascontiguousarray(np.asarray(a, dtype=np.float32))


def _stream_layout(wT, n_out_tiles):
    """wT: [K=1024, M] with M = n_out_tiles*128 -> [n_out_tiles, 128, 1024]
    where tile[mt][:, k*128:(k+1)*128] = wT[k*128:(k+1)*128, mt*128:(mt+1)*128]."""
    K, M = wT.shape
    assert K == H and M == n_out_tiles * 128
    return np.ascontiguousarray(
        wT.reshape(HT, 128, n_out_tiles, 128).transpose(2, 1, 0, 3).reshape(n_out_tiles, 128, HT * 128)
    )


WHH_ORDER_H = [mt for j in range(HT) for mt in (j, 8 + j, 16 + j)]
WHH_POS = {mt: i for i, mt in enumerate(WHH_ORDER_H)}


def _resident_layout(wT, n_out_tiles):
    """Same data as _stream_layout but flattened to [128, 24*1024] with
    column blocks in CONSUMPTION order (j-interleaved, see WHH_ORDER_H)."""
    s = _stream_layout(wT, n_out_tiles)  # [mt, 128, 1024]
    s = s[WHH_ORDER_H]
    return np.ascontiguousarray(s.transpose(1, 0, 2).reshape(128, n_out_tiles * HT * 128))


def _per_part(vec):
    """[n*128] f32 -> [128, n] with column j = chunk j (per-partition scalars)."""
    v = _f32(vec)
    n = v.shape[0] // 128
    return np.ascontiguousarray(v.reshape(n, 128).T)


# ----------------------------------------------------------------------------
# graph builder
# ----------------------------------------------------------------------------

_CACHE = {}


def _build():
    if "nc" in _CACHE:
        return _CACHE["nc"], _CACHE["names"]

    nc = bacc.Bacc("TRN2", target_bir_lowering=False, debug=False,
                   num_devices=N_CORES)

    d = {}
    # per-core activation inputs (bf16, transposed layouts)
    d["xt"] = nc.dram_tensor("xt", [XD, NN * BL], BF16, kind="ExternalInput").ap()
    d["xloopt"] = nc.dram_tensor("xloopt", [XD, NN * BL], BF16, kind="ExternalInput").ap()
    d["xroott"] = nc.dram_tensor("xroott", [X0D, BL], BF16, kind="ExternalInput").ap()
    d["pmask"] = nc.dram_tensor("pmask", [N_PAIRS, 128, BL], BF16, kind="ExternalInput").ap()
    d["smask"] = nc.dram_tensor("smask", [N_PAIRS, 128, BL], BF16, kind="ExternalInput").ap()
    d["pmask8"] = nc.dram_tensor("pmask8", [N_PAIRS, 128, HT * BL], BF16, kind="ExternalInput").ap()
    d["smask8"] = nc.dram_tensor("smask8", [N_PAIRS, 128, HT * BL], BF16, kind="ExternalInput").ap()
    d["bgw"] = nc.dram_tensor("bgw", [128, HT * BL], BF16, kind="ExternalInput").ap()
    # weights (replicated)
    d["whhc_s"] = nc.dram_tensor("whhc_s", [24, 128, HT * 128], BF16, kind="ExternalInput").ap()
    d["whhl_r"] = nc.dram_tensor("whhl_r", [128, 24 * HT * 128], BF16, kind="ExternalInput").ap()
    d["whhr_s"] = nc.dram_tensor("whhr_s", [24, 128, HT * 128], BF16, kind="ExternalInput").ap()
    d["wihc"] = nc.dram_tensor("wihc", [XD, 3 * H], BF16, kind="ExternalInput").ap()
    d["wihl"] = nc.dram_tensor("wihl", [XD, 3 * H], BF16, kind="ExternalInput").ap()
    d["wihr"] = nc.dram_tensor("wihr", [X0D, 3 * H], BF16, kind="ExternalInput").ap()
    d["projt_s"] = nc.dram_tensor("projt_s", [32, 128, HT * 128], BF16, kind="ExternalInput").ap()
    d["wheadt_s"] = nc.dram_tensor("wheadt_s", [4, 128, HT * 128], BF16, kind="ExternalInput").ap()
    # biases (f32 per-partition scalar banks)
    d["brz_c"] = nc.dram_tensor("brz_c", [128, 16], F32, kind="ExternalInput").ap()
    d["brz_l"] = nc.dram_tensor("brz_l", [128, 16], F32, kind="ExternalInput").ap()
    d["brz_r"] = nc.dram_tensor("brz_r", [128, 16], F32, kind="ExternalInput").ap()
    d["bin_c"] = nc.dram_tensor("bin_c", [128, 8], F32, kind="ExternalInput").ap()
    d["bin_l"] = nc.dram_tensor("bin_l", [128, 8], F32, kind="ExternalInput").ap()
    d["bin_r"] = nc.dram_tensor("bin_r", [128, 8], F32, kind="ExternalInput").ap()
    d["bhn_c"] = nc.dram_tensor("bhn_c", [128, 8], F32, kind="ExternalInput").ap()
    d["bhn_l"] = nc.dram_tensor("bhn_l", [128, 8], F32, kind="ExternalInput").ap()
    d["bhn_r"] = nc.dram_tensor("bhn_r", [128, 8], F32, kind="ExternalInput").ap()
    d["bg"] = nc.dram_tensor("bg", [128, 8], F32, kind="ExternalInput").ap()
    d["bmu"] = nc.dram_tensor("bmu", [128, 2], F32, kind="ExternalInput").ap()
    d["bstd"] = nc.dram_tensor("bstd", [128, 2], F32, kind="ExternalInput").ap()
    # outputs
    d["mu"] = nc.dram_tensor("mu", [BL, Z], F32, kind="ExternalOutput").ap()
    d["stdo"] = nc.dram_tensor("stdo", [BL, Z], F32, kind="ExternalOutput").ap()

    with tile.TileContext(nc) as tc, ExitStack() as ctx:
        _emit(ctx, tc, d)

    nc.compile()
    _CACHE["nc"] = nc
    _CACHE["names"] = d
    return nc, d


def _emit(ctx, tc, d):
    nc = tc.nc

    wpool = ctx.enter_context(tc.tile_pool(name="wres", bufs=1))
    wstream = ctx.enter_context(tc.tile_pool(name="wstream", bufs=8))
    pstream = ctx.enter_context(tc.tile_pool(name="pstream", bufs=6))
    accp = ctx.enter_context(tc.tile_pool(name="accp", bufs=1))
    adp = ctx.enter_context(tc.tile_pool(name="adp", bufs=2))
    statep = ctx.enter_context(tc.tile_pool(name="statep", bufs=1))
    smallp = ctx.enter_context(tc.tile_pool(name="smallp", bufs=6))
    widep = ctx.enter_context(tc.tile_pool(name="widep", bufs=3))
    maskp = ctx.enter_context(tc.tile_pool(name="maskp", bufs=4))
    wmaskp = ctx.enter_context(tc.tile_pool(name="wmaskp", bufs=2))
    outp = ctx.enter_context(tc.tile_pool(name="outp", bufs=1))
    psum = ctx.enter_context(tc.tile_pool(name="psum", bufs=6, space="PSUM"))
    psuml = ctx.enter_context(tc.tile_pool(name="psuml", bufs=2, space="PSUM"))

    # ---------------- prologue: small loads first so node 6 starts early ---
    wihc = wpool.tile([XD, 3 * H], BF16)
    nc.sync.dma_start(wihc[:], d["wihc"][:])
    xt = wpool.tile([XD, NN * BL], BF16)
    nc.sync.dma_start(xt[:], d["xt"][:])
    wihl = wpool.tile([XD, 3 * H], BF16)
    nc.scalar.dma_start(wihl[:], d["wihl"][:])
    xloopt = wpool.tile([XD, NN * BL], BF16)
    nc.scalar.dma_start(xloopt[:], d["xloopt"][:])
    wihr = wpool.tile([X0D, 3 * H], BF16)
    nc.scalar.dma_start(wihr[:], d["wihr"][:])
    xroott = wpool.tile([X0D, BL], BF16)
    nc.scalar.dma_start(xroott[:], d["xroott"][:])

    biases = {}
    for nm in ["brz_c", "brz_l", "brz_r", "bin_c", "bin_l", "bin_r",
               "bhn_c", "bhn_l", "bhn_r", "bg", "bmu", "bstd"]:
        t = wpool.tile(list(d[nm].shape), F32, name=nm + "_sb")
        nc.sync.dma_start(t[:], d[nm][:])
        biases[nm] = t

    # big resident Whh_l load, split across both HWDGE rings
    whhl = wpool.tile([128, 24 * HT * 128], BF16, tag="whhlr")
    for q in range(4):
        sl = slice(q * 6 * HT * 128, (q + 1) * 6 * HT * 128)
        eng = nc.sync if q % 2 == 0 else nc.scalar
        eng.dma_start(whhl[:, sl], d["whhl_r"][:, sl])

    bgw = wpool.tile([128, HT * BL], BF16)
    nc.sync.dma_start(bgw[:], d["bgw"][:])

    ident = wpool.tile([128, 128], F32)
    make_identity(nc, ident[:])

    # accumulators for H_in(v), v = 0..5 : [128, HT*BL] bf16, chunk k at col k*BL
    acc = {}
    for v in range(6):
        acc[v] = accp.tile([128, HT * BL], BF16, name=f"acc{v}", tag=f"acc{v}")

    # A..D projection buffer of current node x: [128, 32*BL] bf16 (mt at col mt*BL)
    # mts 0..7 = A, 8..15 = B, 16..23 = C, 24..31 = D
    state = {}

    WHH_ORDER = [mt for j in range(HT) for mt in (j, 8 + j, 16 + j)]

    def emit_whh_stream(vnode, src_name, tiles=None, lo=0, hi=24):
        """DMA [128, 8*128] lhsT tiles of Whh for node vnode's first GRU.
        Emitted in CONSUMPTION order (per j-group: r, z, n tiles) so the DMA
        FIFO order matches the slot-release order -- otherwise the scheduler
        can deadlock on pool-slot WAR with only 12 bufs.  lo/hi select a
        sub-range of the consumption order (for the first node's split)."""
        if tiles is None:
            tiles = [None] * 24
        for mt in WHH_ORDER[lo:hi]:
            wt = wstream.tile([128, HT * 128], BF16, name=f"whh_{vnode}_{mt}", tag="whhc")
            nc.sync.dma_start(wt[:], d[src_name][mt, :, :])
            tiles[mt] = wt
        return tiles

    def gru(vnode, cell, h_in, rhs_x, out_name, first_gru_zero_h=False,
            whh_stream=None):
        """Emit one GRUCell: h' -> state[out_name].
        h_in: AP [128, HT*BL] bf16 (chunk k at col k*BL) or None when h == 0.
        rhs_x: AP [xdim, BL] (bf16) input slice.
        cell: 'c' | 'l' | 'r'.  'c'/'r' read whh_stream tiles, 'l' the resident."""
        wih = {"c": wihc, "l": wihl, "r": wihr}[cell]
        brz = biases[f"brz_{cell}"]
        bin_ = biases[f"bin_{cell}"]
        bhn = biases[f"bhn_{cell}"]
        hv = statep.tile([128, HT * BL], BF16, name=f"{out_name}_{vnode}", tag=out_name)

        def hmm(ps, mt, rhs_h, first):
            """8 K-chunk matmuls accumulating into ps.  first=True if this
            opens the accumulation group (no x-matmul before it)."""
            if cell == "l":
                src, base = whhl, WHH_POS[mt] * HT * 128
            else:
                src, base = whh_stream[mt], 0
            for k in range(HT):
                nc.tensor.matmul(ps[:], src[:, base + k * 128: base + (k + 1) * 128],
                                 rhs_h[:, k * BL:(k + 1) * BL],
                                 start=(first and k == 0), stop=(k == HT - 1))

        for j in range(HT):
            # r gate (out rows j*128..), z gate (rows H + j*128..), n (rows 2H + ..)
            pool_rz = psuml if j == 0 else psum
            tag_rz = "lead" if j == 0 else "mm"
            ps_r = pool_rz.tile([128, BL], F32, name=f"psr_{vnode}_{cell}_{j}", tag=tag_rz)
            ps_z = pool_rz.tile([128, BL], F32, name=f"psz_{vnode}_{cell}_{j}", tag=tag_rz)
            if first_gru_zero_h:
                nc.tensor.matmul(ps_r[:], wih[:, j * 128:(j + 1) * 128], rhs_x,
                                 start=True, stop=True)
                nc.tensor.matmul(ps_z[:], wih[:, H + j * 128: H + (j + 1) * 128], rhs_x,
                                 start=True, stop=True)
            else:
                nc.tensor.matmul(ps_r[:], wih[:, j * 128:(j + 1) * 128], rhs_x,
                                 start=True, stop=False)
                hmm(ps_r, j, h_in, first=False)
                nc.tensor.matmul(ps_z[:], wih[:, H + j * 128: H + (j + 1) * 128], rhs_x,
                                 start=True, stop=False)
                hmm(ps_z, 8 + j, h_in, first=False)
            ps_in = psum.tile([128, BL], F32, name=f"psin_{vnode}_{cell}_{j}", tag="mm")
            nc.tensor.matmul(ps_in[:], wih[:, 2 * H + j * 128: 2 * H + (j + 1) * 128],
                             rhs_x, start=True, stop=True)
            if not first_gru_zero_h:
                ps_hn = psum.tile([128, BL], F32, name=f"pshn_{vnode}_{cell}_{j}", tag="mm")
                hmm(ps_hn, 16 + j, h_in, first=True)

            r_t = smallp.tile([128, BL], BF16, name=f"r_{vnode}_{cell}_{j}", tag="sc")
            nc.scalar.activation(r_t[:], ps_r[:], AF.Sigmoid, bias=brz[:, j:j + 1])
            z_t = smallp.tile([128, BL], BF16, name=f"z_{vnode}_{cell}_{j}", tag="sc")
            nc.scalar.activation(z_t[:], ps_z[:], AF.Sigmoid, bias=brz[:, 8 + j: 9 + j])

            t1 = smallp.tile([128, BL], BF16, name=f"t1_{vnode}_{cell}_{j}", tag="sc")
            if first_gru_zero_h:
                # hn + bhn with h==0 -> bhn ; t1 = bhn * r
                nc.vector.tensor_scalar_mul(t1[:], r_t[:], bhn[:, j:j + 1])
            else:
                # t1 = (ps_hn + bhn) * r
                nc.vector.scalar_tensor_tensor(t1[:], ps_hn[:], bhn[:, j:j + 1], r_t[:],
                                               op0=ALU.add, op1=ALU.mult)
            t2 = smallp.tile([128, BL], BF16, name=f"t2_{vnode}_{cell}_{j}", tag="sc")
            nc.vector.tensor_add(t2[:], t1[:], ps_in[:])
            n_t = smallp.tile([128, BL], BF16, name=f"n_{vnode}_{cell}_{j}", tag="sc")
            nc.scalar.activation(n_t[:], t2[:], AF.Tanh, bias=bin_[:, j:j + 1])

            jsl = slice(j * BL, (j + 1) * BL)
            if first_gru_zero_h:
                # h' = n - z*n
                zn = smallp.tile([128, BL], BF16, name=f"zn_{vnode}_{cell}_{j}", tag="sc")
                nc.vector.tensor_mul(zn[:], z_t[:], n_t[:])
                nc.vector.tensor_sub(hv[:, jsl], n_t[:], zn[:])
            else:
                # h' = n + z*(h - n)
                dt_ = smallp.tile([128, BL], BF16, name=f"d_{vnode}_{cell}_{j}", tag="sc")
                nc.vector.tensor_sub(dt_[:], h_in[:, jsl], n_t[:])
                zd = smallp.tile([128, BL], BF16, name=f"zd_{vnode}_{cell}_{j}", tag="sc")
                nc.vector.tensor_mul(zd[:], z_t[:], dt_[:])
                nc.vector.tensor_add(hv[:, jsl], n_t[:], zd[:])
        state[out_name] = hv
        return hv

    def emit_pair(w, x, eng, first_write, ad):
        """acc_w += sigmoid(p*A + s*B + bg) * (p*C + s*D) for neighbor x of
        node w -- full-width [128, 8*BL] ops (all H chunks at once)."""
        pid = _pair_id(w, x)
        W = HT * BL
        # SWDGE (gpsimd) for the wide masks: its slot waits can't head-of-line
        # block the ACT or SP queues (gpsimd is otherwise idle).
        pm = wmaskp.tile([128, W], BF16, name=f"pm8_{w}_{x}", tag="wmask")
        nc.gpsimd.dma_start(pm[:], d["pmask8"][pid, :, :])
        sm = wmaskp.tile([128, W], BF16, name=f"sm8_{w}_{x}", tag="wmask")
        nc.gpsimd.dma_start(sm[:], d["smask8"][pid, :, :])
        a = acc[w]
        # 3 tiles, in-place chain (9-tile version deadlocks the slot pool)
        t1 = widep.tile([128, W], BF16, name=f"pa_{w}_{x}", tag="pw")
        eng.tensor_mul(t1[:], pm[:], ad[:, 0:W])
        t2 = widep.tile([128, W], BF16, name=f"pb_{w}_{x}", tag="pw")
        eng.tensor_mul(t2[:], sm[:], ad[:, W:2 * W])
        eng.tensor_add(t1[:], t1[:], t2[:])
        eng.tensor_add(t1[:], t1[:], bgw[:])
        g = widep.tile([128, W], BF16, name=f"pg_{w}_{x}", tag="pw")
        nc.scalar.activation(g[:], t1[:], AF.Sigmoid)
        eng.tensor_mul(t2[:], pm[:], ad[:, 2 * W:3 * W])
        eng.tensor_mul(t1[:], sm[:], ad[:, 3 * W:4 * W])
        eng.tensor_add(t2[:], t2[:], t1[:])
        if first_write:
            eng.tensor_mul(a[:], g[:], t2[:])
        else:
            eng.tensor_mul(t2[:], g[:], t2[:])
            eng.tensor_add(a[:], a[:], t2[:])

    # ---------------- main node loop ----------------
    whh_stream = None  # lhsT tiles for the CURRENT node's first GRU (c or r)
    deferred = []      # NC pairs queued for emission inside the next node
    for v in range(NN - 1, 0, -1):
        # GRU_c (for v == 6 the hidden input is zero: x-matmuls only)
        xv = xt[:, v * BL:(v + 1) * BL]
        h_in = acc[v] if v < NN - 1 else None
        gru(v, "c", h_in, xv, "hv1", first_gru_zero_h=(v == NN - 1),
            whh_stream=whh_stream)

        # drain 1 deferred NC pair (its DVE ops overlap gh_l's MMs)
        for item in deferred[:1]:
            emit_pair(item[0], item[1], nc.vector, first_write=item[2], ad=item[3])
        del deferred[:1]

        # GRU_l (self-loop GRU); h = hv1
        xlv = xloopt[:, v * BL:(v + 1) * BL]
        gru(v, "l", state["hv1"], xlv, "hv")

        # drain up to 3 more during the proj phase (DVE has slack there)
        for item in deferred[:3]:
            emit_pair(item[0], item[1], nc.vector, first_write=item[2], ad=item[3])
        del deferred[:3]

        # prefetch next node's Whh stream (before proj DMAs in the sync queue).
        # Exception: at the first node the whh-stream slots beyond the pool
        # size would head-of-line-block the proj DMAs (their WAR release is
        # gh_c(5), which needs proj(6)) -> split: first 12 (slots are free)
        # before proj, the rest after.
        src = "whhc_s" if v - 1 >= 1 else "whhr_s"
        if v < NN - 1:
            whh_stream = emit_whh_stream(v - 1, src)
        else:
            whh_stream = emit_whh_stream(v - 1, src, lo=0, hi=12)

        # projections A..D of h_v (weights streamed just-in-time).
        # mt order (j, 8+j, 16+j, 24+j): chunk j of the CRITICAL pair
        # (v-1, v) is fused into the loop, reading the four proj psums
        # directly (no SBUF staging on its path) -> acc_{v-1} finalizes
        # in lockstep with the proj phase and gh_c(v-1) starts immediately.
        state["ad_prev"] = state.get("ad")
        ad = adp.tile([128, 32 * BL], BF16, name=f"ad_{v}", tag="ad")
        hv = state["hv"]
        pid = _pair_id(v - 1, v)
        cpm = maskp.tile([128, BL], BF16, name=f"pm_{v-1}_{v}", tag="mask")
        nc.scalar.dma_start(cpm[:], d["pmask"][pid, :, :])
        csm = maskp.tile([128, BL], BF16, name=f"sm_{v-1}_{v}", tag="mask")
        nc.scalar.dma_start(csm[:], d["smask"][pid, :, :])
        cacc = acc[v - 1]
        # v == 6: (5,6) is acc_5's only contribution.  v == 5: crit (4,5)
        # runs during proj(5), BEFORE the NC pair (4,6) emitted at node-5
        # end -> the crit pair is acc_4's chronological first write.
        first_write = v >= NN - 2
        for j in range(HT):
            pss = []
            for mt in (j, 8 + j, 16 + j, 24 + j):
                pt = pstream.tile([128, HT * 128], BF16, name=f"proj_{v}_{mt}", tag="proj")
                nc.sync.dma_start(pt[:], d["projt_s"][mt, :, :])
                ps = psum.tile([128, BL], F32, name=f"psp_{v}_{mt}", tag="mm")
                for k in range(HT):
                    nc.tensor.matmul(ps[:], pt[:, k * 128:(k + 1) * 128],
                                     hv[:, k * BL:(k + 1) * BL],
                                     start=(k == 0), stop=(k == HT - 1))
                nc.scalar.activation(ad[:, mt * BL:(mt + 1) * BL], ps[:], AF.Copy)
                pss.append(ps)
            psA, psB, psC, psD = pss
            t1 = smallp.tile([128, BL], BF16, name=f"cpa_{v}_{j}", tag="pw")
            nc.vector.tensor_mul(t1[:], cpm[:], psA[:])
            t2 = smallp.tile([128, BL], BF16, name=f"cpb_{v}_{j}", tag="pw")
            nc.vector.tensor_mul(t2[:], csm[:], psB[:])
            u = smallp.tile([128, BL], BF16, name=f"cpu_{v}_{j}", tag="pw")
            nc.vector.tensor_add(u[:], t1[:], t2[:])
            g = smallp.tile([128, BL], BF16, name=f"cpg_{v}_{j}", tag="pw")
            nc.scalar.activation(g[:], u[:], AF.Sigmoid, bias=biases["bg"][:, j:j + 1])
            m1 = smallp.tile([128, BL], BF16, name=f"cpc_{v}_{j}", tag="pw")
            nc.vector.tensor_mul(m1[:], cpm[:], psC[:])
            m2 = smallp.tile([128, BL], BF16, name=f"cpd_{v}_{j}", tag="pw")
            nc.vector.tensor_mul(m2[:], csm[:], psD[:])
            m = smallp.tile([128, BL], BF16, name=f"cpm2_{v}_{j}", tag="pw")
            nc.vector.tensor_add(m[:], m1[:], m2[:])
            jsl = slice(j * BL, (j + 1) * BL)
            if first_write:
                nc.vector.tensor_mul(cacc[:, jsl], g[:], m[:])
            else:
                gm = smallp.tile([128, BL], BF16, name=f"cpgm_{v}_{j}", tag="pw")
                nc.vector.tensor_mul(gm[:], g[:], m[:])
                nc.vector.tensor_add(cacc[:, jsl], cacc[:, jsl], gm[:])
        state["ad"] = ad

        if v == NN - 1:
            whh_stream = emit_whh_stream(v - 1, src, tiles=whh_stream, lo=12, hi=24)

        # non-critical pairs (w, v+1), w = v-1 .. 0, on DVE (wide ops).
        # Emit only the earliest-deadline one (w = v-1, needed by gh_c(v-1))
        # here; defer the rest into node v-1's interior so their DVE bulk
        # doesn't FIFO-wedge node v-1's GRU combines all at once.
        if v < NN - 1:
            for w in range(v - 1, -1, -1):
                first = (v + 1 == 6) and w != v - 1
                item = (w, v + 1, first, state["ad_prev"])
                if w == v - 1:
                    emit_pair(item[0], item[1], nc.vector, first_write=item[2],
                              ad=item[3])
                else:
                    deferred.append(item)

        if v == 1:
            # prefetch head weights (pstream slots now cycle freely)
            head_tiles = []
            for mt in range(4):
                wt = pstream.tile([128, HT * 128], BF16, name=f"whead_{mt}", tag="proj")
                nc.sync.dma_start(wt[:], d["wheadt_s"][mt, :, :])
                head_tiles.append(wt)
            state["head_tiles"] = head_tiles

    # ---------------- root node (v = 0) ----------------
    for item in deferred:
        emit_pair(item[0], item[1], nc.vector, first_write=item[2], ad=item[3])
    del deferred[:]
    # (pair (0, 1) was fused into proj(1) above)
    gru(0, "r", acc[0], xroott[:], "hv1", whh_stream=whh_stream)
    h0 = state["hv1"]

    # heads: mu = Whead[0:2] @ h0 + bmu ; std = softplus(Whead[2:4] @ h0 + bstd)
    musb = outp.tile([128, 2 * BL], F32)
    stdsb = outp.tile([128, 2 * BL], F32)
    for mt in range(4):
        wt = state["head_tiles"][mt]
        ps = psum.tile([128, BL], F32, name=f"pshead_{mt}", tag="mm")
        for k in range(HT):
            nc.tensor.matmul(ps[:], wt[:, k * 128:(k + 1) * 128],
                             h0[:, k * BL:(k + 1) * BL],
                             start=(k == 0), stop=(k == HT - 1))
        zi = mt % 2
        if mt < 2:
            nc.scalar.activation(musb[:, zi * BL:(zi + 1) * BL], ps[:], AF.Identity,
                                 bias=biases["bmu"][:, zi:zi + 1])
        else:
            # softplus(x) = ln(1 + exp(x)); no softplus ACT table available.
            te = outp.tile([128, BL], F32, name=f"te_{mt}", tag="te", bufs=1)
            nc.scalar.activation(te[:], ps[:], AF.Exp, bias=biases["bstd"][:, zi:zi + 1])
            tp1 = outp.tile([128, BL], F32, name=f"tp1_{mt}", tag="tp1", bufs=1)
            nc.vector.tensor_scalar_add(tp1[:], te[:], 1.0)
            nc.scalar.activation(stdsb[:, zi * BL:(zi + 1) * BL], tp1[:], AF.Ln)

    # transpose [Z, B] -> [B, Z] via PE and DMA out
    for head_sb, out_ap in ((musb, d["mu"]), (stdsb, d["stdo"])):
        for bi in range(2):
            osb = outp.tile([128, Z], F32, name=f"o_{out_ap.tensor.name}_{bi}", tag="osb", bufs=2)
            for zi in range(2):
                tp = psum.tile([128, 128], F32, name=f"tp_{out_ap.tensor.name}_{bi}_{zi}", tag="mm")
                nc.tensor.transpose(tp[:], head_sb[:, zi * BL + bi * 128: zi * BL + (bi + 1) * 128],
                                    ident[:])
                nc.scalar.activation(osb[:, zi * 128:(zi + 1) * 128], tp[:], AF.Copy)
            nc.sync.dma_start(out_ap[bi * 128:(bi + 1) * 128, :], osb[:])


# ----------------------------------------------------------------------------
# host-side prep
# ----------------------------------------------------------------------------

def _prepare_shared(inputs):
    """Weight arrays shared by all cores (already laid out for the kernel)."""
    g = {}
    # GRU weight layouts.  torch GRUCell: W_ih [3H, xd] rows = [r; z; n]
    for cell, wih, whh, bih, bhh in (
        ("c", inputs["W_ih_c"], inputs["W_hh_c"], inputs["b_ih_c"], inputs["b_hh_c"]),
        ("l", inputs["W_ih_l"], inputs["W_hh_l"], inputs["b_ih_l"], inputs["b_hh_l"]),
        ("r", inputs["W_ih_r"], inputs["W_hh_r"], inputs["b_ih_r"], inputs["b_hh_r"]),
    ):
        wihT = _f32(wih).T                       # [xd, 3H]
        g[f"wih{cell}"] = _bf(wihT)
        whhT = _f32(whh).T                       # [H, 3H]
        if cell == "c":
            g["whhc_s"] = _bf(_stream_layout(whhT, 24))
        elif cell == "l":
            g["whhl_r"] = _bf(_resident_layout(whhT, 24))
        else:
            g["whhr_s"] = _bf(_stream_layout(whhT, 24))
        bih = _f32(bih)
        bhh = _f32(bhh)
        brz = (bih + bhh)[: 2 * H]               # r and z gate biases combine
        g[f"brz_{cell}"] = _per_part(brz)        # [128, 16]
        g[f"bin_{cell}"] = _per_part(bih[2 * H:])  # [128, 8]
        g[f"bhn_{cell}"] = _per_part(bhh[2 * H:])  # [128, 8]

    Wg = _f32(inputs["Wg"])                      # [H, 2H]
    Wm = _f32(inputs["Wm"])
    P = np.concatenate([Wg[:, :H], Wg[:, H:], Wm[:, :H], Wm[:, H:]], axis=0)  # [4H, H]
    g["projt_s"] = _bf(_stream_layout(P.T, 32))
    g["bg"] = _per_part(inputs["bg"])
    # wide bg: [128, 8*BL], chunk j cols = bg[j*128 + p] broadcast over batch
    g["bgw"] = _bf(np.repeat(_per_part(inputs["bg"]), BL, axis=1))

    Whead = np.concatenate([_f32(inputs["Wmu"]), _f32(inputs["Wstd"])], axis=0)  # [2Z, H]
    g["wheadt_s"] = _bf(_stream_layout(Whead.T, 4))
    g["bmu"] = _per_part(inputs["bmu"])
    g["bstd"] = _per_part(inputs["bstd"])
    return g


def _prepare_core(inputs, ci):
    b0 = ci * BL
    X = _f32(inputs["X"][b0:b0 + BL])            # [BL, 7, 27]
    adj = np.asarray(inputs["adj"][b0:b0 + BL])  # [BL, 7, 7] int32

    m = {}
    # xt: [27, NN*BL] with column v*BL + b  (transpose(2,1,0) -> [27, 7, BL])
    xt = np.ascontiguousarray(X.transpose(2, 1, 0).reshape(XD, NN * BL))
    m["xt"] = _bf(xt)
    selfloop = (np.einsum("bvv->bv", adj) > 0).astype(np.float32)  # [BL, 7]
    Xl = X * selfloop[:, :, None]
    m["xloopt"] = _bf(np.ascontiguousarray(Xl.transpose(2, 1, 0).reshape(XD, NN * BL)))
    m["xroott"] = _bf(np.ascontiguousarray(X[:, 0, :X0D].T))

    pm = np.zeros((N_PAIRS, 128, BL), np.float32)
    sm = np.zeros((N_PAIRS, 128, BL), np.float32)
    for x in range(1, NN):
        for v in range(x):
            pid = _pair_id(v, x)
            pm[pid, :, :] = (adj[:, x, v] > 0).astype(np.float32)[None, :]
            sm[pid, :, :] = (adj[:, v, x] > 0).astype(np.float32)[None, :]
    m["pmask"] = _bf(pm)
    m["smask"] = _bf(sm)
    m["pmask8"] = _bf(np.tile(pm, (1, 1, 8)))
    m["smask8"] = _bf(np.tile(sm, (1, 1, 8)))
    return m


def make_in_maps(inputs):
    shared = _prepare_shared(inputs)
    in_maps = []
    for ci in range(N_CORES):
        mm = dict(shared)
        mm.update(_prepare_core(inputs, ci))
        in_maps.append(mm)
    return in_maps


def run_raw(inputs, trace=False, **kw):
    nc, _ = _build()
    in_maps = make_in_maps(inputs)
    res = run_bass_kernel_spmd(nc, in_maps, core_ids=list(range(N_CORES)),
                               trace=trace, **kw)
    mu = np.concatenate([res.results[ci]["mu"] for ci in range(N_CORES)], axis=0)
    std = np.concatenate([res.results[ci]["stdo"] for ci in range(N_CORES)], axis=0)
    return mu.astype(np.float32), std.astype(np.float32), res


def kernel(**inputs):
    mu, std, _ = run_raw(inputs)
    return mu, std


if __name__ == "__main__":
    nc, _ = _build()
    print("build + compile OK")


# revision 40
# speedup vs baseline: 1.0409x; 1.0409x over previous
"""Trainium2 Bass kernel for nn_DXVAE (GNN message-passing encoder).

Contract: kernel(**inputs) takes the FULL (unsharded) numpy inputs of
reference.setup_inputs() and returns the full (mu, std) outputs.

Strategy: pure data parallelism over the batch (2048 graphs -> 256 per
NeuronCore x 8 cores), weights replicated. Inside each core the per-node
sequential loop (v = 6..0) runs with activations in transposed layout
[H on partitions, batch on free dim].

Key algebraic restructure vs the reference: the reference computes, for
every node v, gate/mapper matmuls over the stacked masked neighbors
(Hcat [B,k,2H] @ [2H,H]).  Because the masks are per-(batch,pair) scalars,
   sigmoid(Hcat @ Wg.T + bg) = sigmoid(p*A_x + s*B_x + bg)
with A_x = Wg[:, :H] @ h_x, B_x = Wg[:, H:] @ h_x (and C_x, D_x for Wm).
So we compute the four projections ONCE per node x (4H^2 MACs) and each
(v, x) pair becomes cheap elementwise work accumulated eagerly into
per-v accumulators.  This cuts the dominant matmul FLOPs ~3.5x.
"""

import sys
import numpy as np

sys.path.insert(0, "/opt/trn_rl_repo")

import ml_dtypes
from contextlib import ExitStack

import concourse.bass as bass
import concourse.mybir as mybir
import concourse.tile as tile
from concourse import bacc
from concourse.bass_utils import run_bass_kernel_spmd
from concourse.masks import make_identity

F32 = mybir.dt.float32
BF16 = mybir.dt.bfloat16
AF = mybir.ActivationFunctionType
ALU = mybir.AluOpType

N_CORES = 8
B = 2048
BL = B // N_CORES          # 256 batch per core
NN = 7                     # nodes
H = 1024
HT = H // 128              # 8 H chunks
Z = 256
XD = 27
X0D = 23

N_PAIRS = (NN - 1) * NN // 2 - 0  # pairs (v, x) with 0 <= v < x <= 6, v<=5 -> 21


def _pair_id(v, x):
    # enumerate pairs (v, x), v < x, ordered
    pid = 0
    for xx in range(1, NN):
        for vv in range(xx):
            if vv == v and xx == x:
                return pid
            pid += 1
    raise ValueError((v, x))


def _bf(a):
    return np.ascontiguousarray(np.asarray(a, dtype=np.float32)).astype(ml_dtypes.bfloat16)


def _f32(a):
    return np.ascontiguousarray(np.asarray(a, dtype=np.float32))


def _stream_layout(wT, n_out_tiles):
    """wT: [K=1024, M] with M = n_out_tiles*128 -> [n_out_tiles, 128, 1024]
    where tile[mt][:, k*128:(k+1)*128] = wT[k*128:(k+1)*128, mt*128:(mt+1)*128]."""
    K, M = wT.shape
    assert K == H and M == n_out_tiles * 128
    return np.ascontiguousarray(
        wT.reshape(HT, 128, n_out_tiles, 128).transpose(2, 1, 0, 3).reshape(n_out_tiles, 128, HT * 128)
    )


WHH_ORDER_H = [mt for j in range(HT) for mt in (j, 8 + j, 16 + j)]
WHH_POS = {mt: i for i, mt in enumerate(WHH_ORDER_H)}


def _resident_layout(wT, n_out_tiles):
    """Same data as _stream_layout but flattened to [128, 24*1024] with
    column blocks in CONSUMPTION order (j-interleaved, see WHH_ORDER_H)."""
    s = _stream_layout(wT, n_out_tiles)  # [mt, 128, 1024]
    s = s[WHH_ORDER_H]
    return np.ascontiguousarray(s.transpose(1, 0, 2).reshape(128, n_out_tiles * HT * 128))


def _per_part(vec):
    """[n*128] f32 -> [128, n] with column j = chunk j (per-partition scalars)."""
    v = _f32(vec)
    n = v.shape[0] // 128
    return np.ascontiguousarray(v.reshape(n, 128).T)


# ----------------------------------------------------------------------------
# graph builder
# ----------------------------------------------------------------------------

_CACHE = {}


def _build():
    if "nc" in _CACHE:
        return _CACHE["nc"], _CACHE["names"]

    nc = bacc.Bacc("TRN2", target_bir_lowering=False, debug=False,
                   num_devices=N_CORES)

    d = {}
    # per-core activation inputs (bf16, transposed layouts)
    d["xt"] = nc.dram_tensor("xt", [XD, NN * BL], BF16, kind="ExternalInput").ap()
    d["xloopt"] = nc.dram_tensor("xloopt", [XD, NN * BL], BF16, kind="ExternalInput").ap()
    d["xroott"] = nc.dram_tensor("xroott", [X0D, BL], BF16, kind="ExternalInput").ap()
    d["pmask"] = nc.dram_tensor("pmask", [N_PAIRS, 128, BL], BF16, kind="ExternalInput").ap()
    d["smask"] = nc.dram_tensor("smask", [N_PAIRS, 128, BL], BF16, kind="ExternalInput").ap()
    d["pmask8"] = nc.dram_tensor("pmask8", [N_PAIRS, 128, HT * BL], BF16, kind="ExternalInput").ap()
    d["smask8"] = nc.dram_tensor("smask8", [N_PAIRS, 128, HT * BL], BF16, kind="ExternalInput").ap()
    d["bgw"] = nc.dram_tensor("bgw", [128, HT * BL], BF16, kind="ExternalInput").ap()
    # weights (replicated)
    d["whhc_s"] = nc.dram_tensor("whhc_s", [24, 128, HT * 128], BF16, kind="ExternalInput").ap()
    d["whhl_r"] = nc.dram_tensor("whhl_r", [128, 24 * HT * 128], BF16, kind="ExternalInput").ap()
    d["whhr_s"] = nc.dram_tensor("whhr_s", [24, 128, HT * 128], BF16, kind="ExternalInput").ap()
    d["wihc"] = nc.dram_tensor("wihc", [XD, 3 * H], BF16, kind="ExternalInput").ap()
    d["wihl"] = nc.dram_tensor("wihl", [XD, 3 * H], BF16, kind="ExternalInput").ap()
    d["wihr"] = nc.dram_tensor("wihr", [X0D, 3 * H], BF16, kind="ExternalInput").ap()
    d["projt_s"] = nc.dram_tensor("projt_s", [32, 128, HT * 128], BF16, kind="ExternalInput").ap()
    d["wheadt_s"] = nc.dram_tensor("wheadt_s", [4, 128, HT * 128], BF16, kind="ExternalInput").ap()
    # biases (f32 per-partition scalar banks)
    d["brz_c"] = nc.dram_tensor("brz_c", [128, 16], F32, kind="ExternalInput").ap()
    d["brz_l"] = nc.dram_tensor("brz_l", [128, 16], F32, kind="ExternalInput").ap()
    d["brz_r"] = nc.dram_tensor("brz_r", [128, 16], F32, kind="ExternalInput").ap()
    d["bin_c"] = nc.dram_tensor("bin_c", [128, 8], F32, kind="ExternalInput").ap()
    d["bin_l"] = nc.dram_tensor("bin_l", [128, 8], F32, kind="ExternalInput").ap()
    d["bin_r"] = nc.dram_tensor("bin_r", [128, 8], F32, kind="ExternalInput").ap()
    d["bhn_c"] = nc.dram_tensor("bhn_c", [128, 8], F32, kind="ExternalInput").ap()
    d["bhn_l"] = nc.dram_tensor("bhn_l", [128, 8], F32, kind="ExternalInput").ap()
    d["bhn_r"] = nc.dram_tensor("bhn_r", [128, 8], F32, kind="ExternalInput").ap()
    d["bg"] = nc.dram_tensor("bg", [128, 8], F32, kind="ExternalInput").ap()
    d["bmu"] = nc.dram_tensor("bmu", [128, 2], F32, kind="ExternalInput").ap()
    d["bstd"] = nc.dram_tensor("bstd", [128, 2], F32, kind="ExternalInput").ap()
    # outputs
    d["mu"] = nc.dram_tensor("mu", [BL, Z], F32, kind="ExternalOutput").ap()
    d["stdo"] = nc.dram_tensor("stdo", [BL, Z], F32, kind="ExternalOutput").ap()

    with tile.TileContext(nc) as tc, ExitStack() as ctx:
        _emit(ctx, tc, d)

    nc.compile()
    _CACHE["nc"] = nc
    _CACHE["names"] = d
    return nc, d


def _emit(ctx, tc, d):
    nc = tc.nc

    wpool = ctx.enter_context(tc.tile_pool(name="wres", bufs=1))
    wstream = ctx.enter_context(tc.tile_pool(name="wstream", bufs=8))
    pstream = ctx.enter_context(tc.tile_pool(name="pstream", bufs=6))
    accp = ctx.enter_context(tc.tile_pool(name="accp", bufs=1))
    adp = ctx.enter_context(tc.tile_pool(name="adp", bufs=2))
    statep = ctx.enter_context(tc.tile_pool(name="statep", bufs=1))
    smallp = ctx.enter_context(tc.tile_pool(name="smallp", bufs=6))
    widep = ctx.enter_context(tc.tile_pool(name="widep", bufs=3))
    maskp = ctx.enter_context(tc.tile_pool(name="maskp", bufs=4))
    wmaskp = ctx.enter_context(tc.tile_pool(name="wmaskp", bufs=2))
    outp = ctx.enter_context(tc.tile_pool(name="outp", bufs=1))
    psum = ctx.enter_context(tc.tile_pool(name="psum", bufs=6, space="PSUM"))
    psuml = ctx.enter_context(tc.tile_pool(name="psuml", bufs=2, space="PSUM"))

    # ---------------- prologue: small loads first so node 6 starts early ---
    wihc = wpool.tile([XD, 3 * H], BF16)
    nc.sync.dma_start(wihc[:], d["wihc"][:])
    xt = wpool.tile([XD, NN * BL], BF16)
    nc.sync.dma_start(xt[:], d["xt"][:])
    wihl = wpool.tile([XD, 3 * H], BF16)
    nc.scalar.dma_start(wihl[:], d["wihl"][:])
    xloopt = wpool.tile([XD, NN * BL], BF16)
    nc.scalar.dma_start(xloopt[:], d["xloopt"][:])
    wihr = wpool.tile([X0D, 3 * H], BF16)
    nc.scalar.dma_start(wihr[:], d["wihr"][:])
    xroott = wpool.tile([X0D, BL], BF16)
    nc.scalar.dma_start(xroott[:], d["xroott"][:])

    biases = {}
    for nm in ["brz_c", "brz_l", "brz_r", "bin_c", "bin_l", "bin_r",
               "bhn_c", "bhn_l", "bhn_r", "bg", "bmu", "bstd"]:
        t = wpool.tile(list(d[nm].shape), F32, name=nm + "_sb")
        nc.sync.dma_start(t[:], d[nm][:])
        biases[nm] = t

    # big resident Whh_l load, split across both HWDGE rings
    whhl = wpool.tile([128, 24 * HT * 128], BF16, tag="whhlr")
    for q in range(4):
        sl = slice(q * 6 * HT * 128, (q + 1) * 6 * HT * 128)
        eng = nc.sync if q % 2 == 0 else nc.scalar
        eng.dma_start(whhl[:, sl], d["whhl_r"][:, sl])

    bgw = wpool.tile([128, HT * BL], BF16)
    nc.sync.dma_start(bgw[:], d["bgw"][:])

    ident = wpool.tile([128, 128], F32)
    make_identity(nc, ident[:])

    # accumulators for H_in(v), v = 0..5 : [128, HT*BL] bf16, chunk k at col k*BL
    acc = {}
    for v in range(6):
        acc[v] = accp.tile([128, HT * BL], BF16, name=f"acc{v}", tag=f"acc{v}")

    # A..D projection buffer of current node x: [128, 32*BL] bf16 (mt at col mt*BL)
    # mts 0..7 = A, 8..15 = B, 16..23 = C, 24..31 = D
    state = {}

    WHH_ORDER = [mt for j in range(HT) for mt in (j, 8 + j, 16 + j)]

    def emit_whh_stream(vnode, src_name, tiles=None, lo=0, hi=24):
        """DMA [128, 8*128] lhsT tiles of Whh for node vnode's first GRU.
        Emitted in CONSUMPTION order (per j-group: r, z, n tiles) so the DMA
        FIFO order matches the slot-release order -- otherwise the scheduler
        can deadlock on pool-slot WAR with only 12 bufs.  lo/hi select a
        sub-range of the consumption order (for the first node's split)."""
        if tiles is None:
            tiles = [None] * 24
        for mt in WHH_ORDER[lo:hi]:
            wt = wstream.tile([128, HT * 128], BF16, name=f"whh_{vnode}_{mt}", tag="whhc")
            nc.sync.dma_start(wt[:], d[src_name][mt, :, :])
            tiles[mt] = wt
        return tiles

    def gru(vnode, cell, h_in, rhs_x, out_name, first_gru_zero_h=False,
            whh_stream=None):
        """Emit one GRUCell: h' -> state[out_name].
        h_in: AP [128, HT*BL] bf16 (chunk k at col k*BL) or None when h == 0.
        rhs_x: AP [xdim, BL] (bf16) input slice.
        cell: 'c' | 'l' | 'r'.  'c'/'r' read whh_stream tiles, 'l' the resident."""
        wih = {"c": wihc, "l": wihl, "r": wihr}[cell]
        brz = biases[f"brz_{cell}"]
        bin_ = biases[f"bin_{cell}"]
        bhn = biases[f"bhn_{cell}"]
        hv = statep.tile([128, HT * BL], BF16, name=f"{out_name}_{vnode}", tag=out_name)

        def hmm(ps, mt, rhs_h, first):
            """8 K-chunk matmuls accumulating into ps.  first=True if this
            opens the accumulation group (no x-matmul before it)."""
            if cell == "l":
                src, base = whhl, WHH_POS[mt] * HT * 128
            else:
                src, base = whh_stream[mt], 0
            for k in range(HT):
                nc.tensor.matmul(ps[:], src[:, base + k * 128: base + (k + 1) * 128],
                                 rhs_h[:, k * BL:(k + 1) * BL],
                                 start=(first and k == 0), stop=(k == HT - 1))

        for j in range(HT):
            # r gate (out rows j*128..), z gate (rows H + j*128..), n (rows 2H + ..)
            pool_rz = psuml if j == 0 else psum
            tag_rz = "lead" if j == 0 else "mm"
            ps_r = pool_rz.tile([128, BL], F32, name=f"psr_{vnode}_{cell}_{j}", tag=tag_rz)
            ps_z = pool_rz.tile([128, BL], F32, name=f"psz_{vnode}_{cell}_{j}", tag=tag_rz)
            if first_gru_zero_h:
                nc.tensor.matmul(ps_r[:], wih[:, j * 128:(j + 1) * 128], rhs_x,
                                 start=True, stop=True)
                nc.tensor.matmul(ps_z[:], wih[:, H + j * 128: H + (j + 1) * 128], rhs_x,
                                 start=True, stop=True)
            else:
                nc.tensor.matmul(ps_r[:], wih[:, j * 128:(j + 1) * 128], rhs_x,
                                 start=True, stop=False)
                hmm(ps_r, j, h_in, first=False)
                nc.tensor.matmul(ps_z[:], wih[:, H + j * 128: H + (j + 1) * 128], rhs_x,
                                 start=True, stop=False)
                hmm(ps_z, 8 + j, h_in, first=False)
            ps_in = psum.tile([128, BL], F32, name=f"psin_{vnode}_{cell}_{j}", tag="mm")
            nc.tensor.matmul(ps_in[:], wih[:, 2 * H + j * 128: 2 * H + (j + 1) * 128],
                             rhs_x, start=True, stop=True)
            if not first_gru_zero_h:
                ps_hn = psum.tile([128, BL], F32, name=f"pshn_{vnode}_{cell}_{j}", tag="mm")
                hmm(ps_hn, 16 + j, h_in, first=True)

            r_t = smallp.tile([128, BL], BF16, name=f"r_{vnode}_{cell}_{j}", tag="sc")
            nc.scalar.activation(r_t[:], ps_r[:], AF.Sigmoid, bias=brz[:, j:j + 1])
            z_t = smallp.tile([128, BL], BF16, name=f"z_{vnode}_{cell}_{j}", tag="sc")
            nc.scalar.activation(z_t[:], ps_z[:], AF.Sigmoid, bias=brz[:, 8 + j: 9 + j])

            t1 = smallp.tile([128, BL], BF16, name=f"t1_{vnode}_{cell}_{j}", tag="sc")
            if first_gru_zero_h:
                # hn + bhn with h==0 -> bhn ; t1 = bhn * r
                nc.vector.tensor_scalar_mul(t1[:], r_t[:], bhn[:, j:j + 1])
            else:
                # t1 = (ps_hn + bhn) * r
                nc.vector.scalar_tensor_tensor(t1[:], ps_hn[:], bhn[:, j:j + 1], r_t[:],
                                               op0=ALU.add, op1=ALU.mult)
            t2 = smallp.tile([128, BL], BF16, name=f"t2_{vnode}_{cell}_{j}", tag="sc")
            nc.vector.tensor_add(t2[:], t1[:], ps_in[:])
            n_t = smallp.tile([128, BL], BF16, name=f"n_{vnode}_{cell}_{j}", tag="sc")
            nc.scalar.activation(n_t[:], t2[:], AF.Tanh, bias=bin_[:, j:j + 1])

            jsl = slice(j * BL, (j + 1) * BL)
            if first_gru_zero_h:
                # h' = n - z*n
                zn = smallp.tile([128, BL], BF16, name=f"zn_{vnode}_{cell}_{j}", tag="sc")
                nc.vector.tensor_mul(zn[:], z_t[:], n_t[:])
                nc.vector.tensor_sub(hv[:, jsl], n_t[:], zn[:])
            else:
                # h' = n + z*(h - n)
                dt_ = smallp.tile([128, BL], BF16, name=f"d_{vnode}_{cell}_{j}", tag="sc")
                nc.vector.tensor_sub(dt_[:], h_in[:, jsl], n_t[:])
                zd = smallp.tile([128, BL], BF16, name=f"zd_{vnode}_{cell}_{j}", tag="sc")
                nc.vector.tensor_mul(zd[:], z_t[:], dt_[:])
                nc.vector.tensor_add(hv[:, jsl], n_t[:], zd[:])
        state[out_name] = hv
        return hv

    def emit_pair(w, x, eng, first_write, ad):
        """acc_w += sigmoid(p*A + s*B + bg) * (p*C + s*D) for neighbor x of
        node w -- full-width [128, 8*BL] ops (all H chunks at once)."""
        pid = _pair_id(w, x)
        W = HT * BL
        # SWDGE (gpsimd) for the wide masks: its slot waits can't head-of-line
        # block the ACT or SP queues (gpsimd is otherwise idle).
        pm = wmaskp.tile([128, W], BF16, name=f"pm8_{w}_{x}", tag="wmask")
        nc.gpsimd.dma_start(pm[:], d["pmask8"][pid, :, :])
        sm = wmaskp.tile([128, W], BF16, name=f"sm8_{w}_{x}", tag="wmask")
        nc.gpsimd.dma_start(sm[:], d["smask8"][pid, :, :])
        a = acc[w]
        # 3 tiles, in-place chain (9-tile version deadlocks the slot pool)
        t1 = widep.tile([128, W], BF16, name=f"pa_{w}_{x}", tag="pw")
        eng.tensor_mul(t1[:], pm[:], ad[:, 0:W])
        t2 = widep.tile([128, W], BF16, name=f"pb_{w}_{x}", tag="pw")
        eng.tensor_mul(t2[:], sm[:], ad[:, W:2 * W])
        eng.tensor_add(t1[:], t1[:], t2[:])
        eng.tensor_add(t1[:], t1[:], bgw[:])
        g = widep.tile([128, W], BF16, name=f"pg_{w}_{x}", tag="pw")
        nc.scalar.activation(g[:], t1[:], AF.Sigmoid)
        eng.tensor_mul(t2[:], pm[:], ad[:, 2 * W:3 * W])
        eng.tensor_mul(t1[:], sm[:], ad[:, 3 * W:4 * W])
        eng.tensor_add(t2[:], t2[:], t1[:])
        if first_write:
            eng.tensor_mul(a[:], g[:], t2[:])
        else:
            eng.tensor_mul(t2[:], g[:], t2[:])
            eng.tensor_add(a[:], a[:], t2[:])

    # ---------------- main node loop ----------------
    whh_stream = None  # lhsT tiles for the CURRENT node's first GRU (c or r)
    deferred = []      # NC pairs queued for emission inside the next node
    for v in range(NN - 1, 0, -1):
        # GRU_c (for v == 6 the hidden input is zero: x-matmuls only)
        xv = xt[:, v * BL:(v + 1) * BL]
        h_in = acc[v] if v < NN - 1 else None
        gru(v, "c", h_in, xv, "hv1", first_gru_zero_h=(v == NN - 1),
            whh_stream=whh_stream)

        # drain 1 deferred NC pair (its DVE ops overlap gh_l's MMs)
        for item in deferred[:1]:
            emit_pair(item[0], item[1], nc.vector, first_write=item[2], ad=item[3])
        del deferred[:1]

        # GRU_l (self-loop GRU); h = hv1
        xlv = xloopt[:, v * BL:(v + 1) * BL]
        gru(v, "l", state["hv1"], xlv, "hv")

        # drain up to 3 more during the proj phase (DVE has slack there)
        for item in deferred[:3]:
            emit_pair(item[0], item[1], nc.vector, first_write=item[2], ad=item[3])
        del deferred[:3]

        # immediate NC pair (v-1, v+1): for v <= 4 its write order vs the
        # fused crit pair (v-1, v) is free (neither is acc's first write), so
        # emit it BEFORE proj(v) -- its DVE ops then overlap gh_l/proj here
        # instead of stalling the next node's gh_c.
        if v <= NN - 3 and v >= 1 and v < NN - 1:
            emit_pair(v - 1, v + 1, nc.vector, first_write=False, ad=state["ad"])

        # prefetch next node's Whh stream (before proj DMAs in the sync queue).
        # Exception: at the first node the whh-stream slots beyond the pool
        # size would head-of-line-block the proj DMAs (their WAR release is
        # gh_c(5), which needs proj(6)) -> split: first 12 (slots are free)
        # before proj, the rest after.
        src = "whhc_s" if v - 1 >= 1 else "whhr_s"
        if v < NN - 1:
            whh_stream = emit_whh_stream(v - 1, src)
        else:
            whh_stream = emit_whh_stream(v - 1, src, lo=0, hi=12)

        # projections A..D of h_v (weights streamed just-in-time).
        # mt order (j, 8+j, 16+j, 24+j): chunk j of the CRITICAL pair
        # (v-1, v) is fused into the loop, reading the four proj psums
        # directly (no SBUF staging on its path) -> acc_{v-1} finalizes
        # in lockstep with the proj phase and gh_c(v-1) starts immediately.
        state["ad_prev"] = state.get("ad")
        ad = adp.tile([128, 32 * BL], BF16, name=f"ad_{v}", tag="ad")
        hv = state["hv"]
        pid = _pair_id(v - 1, v)
        cpm = maskp.tile([128, BL], BF16, name=f"pm_{v-1}_{v}", tag="mask")
        nc.scalar.dma_start(cpm[:], d["pmask"][pid, :, :])
        csm = maskp.tile([128, BL], BF16, name=f"sm_{v-1}_{v}", tag="mask")
        nc.scalar.dma_start(csm[:], d["smask"][pid, :, :])
        cacc = acc[v - 1]
        # v == 6: (5,6) is acc_5's only contribution.  v == 5: crit (4,5)
        # runs during proj(5), BEFORE the NC pair (4,6) emitted at node-5
        # end -> the crit pair is acc_4's chronological first write.
        first_write = v >= NN - 2
        for j in range(HT):
            pss = []
            for mt in (j, 8 + j, 16 + j, 24 + j):
                pt = pstream.tile([128, HT * 128], BF16, name=f"proj_{v}_{mt}", tag="proj")
                nc.sync.dma_start(pt[:], d["projt_s"][mt, :, :])
                ps = psum.tile([128, BL], F32, name=f"psp_{v}_{mt}", tag="mm")
                for k in range(HT):
                    nc.tensor.matmul(ps[:], pt[:, k * 128:(k + 1) * 128],
                                     hv[:, k * BL:(k + 1) * BL],
                                     start=(k == 0), stop=(k == HT - 1))
                nc.scalar.activation(ad[:, mt * BL:(mt + 1) * BL], ps[:], AF.Copy)
                pss.append(ps)
            psA, psB, psC, psD = pss
            t1 = smallp.tile([128, BL], BF16, name=f"cpa_{v}_{j}", tag="pw")
            nc.vector.tensor_mul(t1[:], cpm[:], psA[:])
            t2 = smallp.tile([128, BL], BF16, name=f"cpb_{v}_{j}", tag="pw")
            nc.vector.tensor_mul(t2[:], csm[:], psB[:])
            u = smallp.tile([128, BL], BF16, name=f"cpu_{v}_{j}", tag="pw")
            nc.vector.tensor_add(u[:], t1[:], t2[:])
            g = smallp.tile([128, BL], BF16, name=f"cpg_{v}_{j}", tag="pw")
            nc.scalar.activation(g[:], u[:], AF.Sigmoid, bias=biases["bg"][:, j:j + 1])
            m1 = smallp.tile([128, BL], BF16, name=f"cpc_{v}_{j}", tag="pw")
            nc.vector.tensor_mul(m1[:], cpm[:], psC[:])
            m2 = smallp.tile([128, BL], BF16, name=f"cpd_{v}_{j}", tag="pw")
            nc.vector.tensor_mul(m2[:], csm[:], psD[:])
            m = smallp.tile([128, BL], BF16, name=f"cpm2_{v}_{j}", tag="pw")
            nc.vector.tensor_add(m[:], m1[:], m2[:])
            jsl = slice(j * BL, (j + 1) * BL)
            if first_write:
                nc.vector.tensor_mul(cacc[:, jsl], g[:], m[:])
            else:
                gm = smallp.tile([128, BL], BF16, name=f"cpgm_{v}_{j}", tag="pw")
                nc.vector.tensor_mul(gm[:], g[:], m[:])
                nc.vector.tensor_add(cacc[:, jsl], cacc[:, jsl], gm[:])
        state["ad"] = ad

        if v == NN - 1:
            whh_stream = emit_whh_stream(v - 1, src, tiles=whh_stream, lo=12, hi=24)

        # non-critical pairs (w, v+1), w = v-1 .. 0, on DVE (wide ops).
        # Emit only the earliest-deadline one (w = v-1, needed by gh_c(v-1))
        # here; defer the rest into node v-1's interior so their DVE bulk
        # doesn't FIFO-wedge node v-1's GRU combines all at once.
        if v < NN - 1:
            for w in range(v - 1, -1, -1):
                first = (v + 1 == 6) and w != v - 1
                item = (w, v + 1, first, state["ad_prev"])
                if w == v - 1:
                    if v == NN - 2:
                        # (4,6) must follow crit (4,5)'s first write -> here
                        emit_pair(item[0], item[1], nc.vector,
                                  first_write=item[2], ad=item[3])
                    # for v <= 4 it was already emitted before proj(v)
                else:
                    deferred.append(item)

        if v == 1:
            # prefetch head weights (pstream slots now cycle freely)
            head_tiles = []
            for mt in range(4):
                wt = pstream.tile([128, HT * 128], BF16, name=f"whead_{mt}", tag="proj")
                nc.sync.dma_start(wt[:], d["wheadt_s"][mt, :, :])
                head_tiles.append(wt)
            state["head_tiles"] = head_tiles

    # ---------------- root node (v = 0) ----------------
    for item in deferred:
        emit_pair(item[0], item[1], nc.vector, first_write=item[2], ad=item[3])
    del deferred[:]
    # (pair (0, 1) was fused into proj(1) above)
    gru(0, "r", acc[0], xroott[:], "hv1", whh_stream=whh_stream)
    h0 = state["hv1"]

    # heads: mu = Whead[0:2] @ h0 + bmu ; std = softplus(Whead[2:4] @ h0 + bstd)
    musb = outp.tile([128, 2 * BL], F32)
    stdsb = outp.tile([128, 2 * BL], F32)
    for mt in range(4):
        wt = state["head_tiles"][mt]
        ps = psum.tile([128, BL], F32, name=f"pshead_{mt}", tag="mm")
        for k in range(HT):
            nc.tensor.matmul(ps[:], wt[:, k * 128:(k + 1) * 128],
                             h0[:, k * BL:(k + 1) * BL],
                             start=(k == 0), stop=(k == HT - 1))
        zi = mt % 2
        if mt < 2:
            nc.scalar.activation(musb[:, zi * BL:(zi + 1) * BL], ps[:], AF.Identity,
                                 bias=biases["bmu"][:, zi:zi + 1])
        else:
            # softplus(x) = ln(1 + exp(x)); no softplus ACT table available.
            te = outp.tile([128, BL], F32, name=f"te_{mt}", tag="te", bufs=1)
            nc.scalar.activation(te[:], ps[:], AF.Exp, bias=biases["bstd"][:, zi:zi + 1])
            tp1 = outp.tile([128, BL], F32, name=f"tp1_{mt}", tag="tp1", bufs=1)
            nc.vector.tensor_scalar_add(tp1[:], te[:], 1.0)
            nc.scalar.activation(stdsb[:, zi * BL:(zi + 1) * BL], tp1[:], AF.Ln)

    # transpose [Z, B] -> [B, Z] via PE and DMA out
    for head_sb, out_ap in ((musb, d["mu"]), (stdsb, d["stdo"])):
        for bi in range(2):
            osb = outp.tile([128, Z], F32, name=f"o_{out_ap.tensor.name}_{bi}", tag="osb", bufs=2)
            for zi in range(2):
                tp = psum.tile([128, 128], F32, name=f"tp_{out_ap.tensor.name}_{bi}_{zi}", tag="mm")
                nc.tensor.transpose(tp[:], head_sb[:, zi * BL + bi * 128: zi * BL + (bi + 1) * 128],
                                    ident[:])
                nc.scalar.activation(osb[:, zi * 128:(zi + 1) * 128], tp[:], AF.Copy)
            nc.sync.dma_start(out_ap[bi * 128:(bi + 1) * 128, :], osb[:])


# ----------------------------------------------------------------------------
# host-side prep
# ----------------------------------------------------------------------------

def _prepare_shared(inputs):
    """Weight arrays shared by all cores (already laid out for the kernel)."""
    g = {}
    # GRU weight layouts.  torch GRUCell: W_ih [3H, xd] rows = [r; z; n]
    for cell, wih, whh, bih, bhh in (
        ("c", inputs["W_ih_c"], inputs["W_hh_c"], inputs["b_ih_c"], inputs["b_hh_c"]),
        ("l", inputs["W_ih_l"], inputs["W_hh_l"], inputs["b_ih_l"], inputs["b_hh_l"]),
        ("r", inputs["W_ih_r"], inputs["W_hh_r"], inputs["b_ih_r"], inputs["b_hh_r"]),
    ):
        wihT = _f32(wih).T                       # [xd, 3H]
        g[f"wih{cell}"] = _bf(wihT)
        whhT = _f32(whh).T                       # [H, 3H]
        if cell == "c":
            g["whhc_s"] = _bf(_stream_layout(whhT, 24))
        elif cell == "l":
            g["whhl_r"] = _bf(_resident_layout(whhT, 24))
        else:
            g["whhr_s"] = _bf(_stream_layout(whhT, 24))
        bih = _f32(bih)
        bhh = _f32(bhh)
        brz = (bih + bhh)[: 2 * H]               # r and z gate biases combine
        g[f"brz_{cell}"] = _per_part(brz)        # [128, 16]
        g[f"bin_{cell}"] = _per_part(bih[2 * H:])  # [128, 8]
        g[f"bhn_{cell}"] = _per_part(bhh[2 * H:])  # [128, 8]

    Wg = _f32(inputs["Wg"])                      # [H, 2H]
    Wm = _f32(inputs["Wm"])
    P = np.concatenate([Wg[:, :H], Wg[:, H:], Wm[:, :H], Wm[:, H:]], axis=0)  # [4H, H]
    g["projt_s"] = _bf(_stream_layout(P.T, 32))
    g["bg"] = _per_part(inputs["bg"])
    # wide bg: [128, 8*BL], chunk j cols = bg[j*128 + p] broadcast over batch
    g["bgw"] = _bf(np.repeat(_per_part(inputs["bg"]), BL, axis=1))

    Whead = np.concatenate([_f32(inputs["Wmu"]), _f32(inputs["Wstd"])], axis=0)  # [2Z, H]
    g["wheadt_s"] = _bf(_stream_layout(Whead.T, 4))
    g["bmu"] = _per_part(inputs["bmu"])
    g["bstd"] = _per_part(inputs["bstd"])
    return g


def _prepare_core(inputs, ci):
    b0 = ci * BL
    X = _f32(inputs["X"][b0:b0 + BL])            # [BL, 7, 27]
    adj = np.asarray(inputs["adj"][b0:b0 + BL])  # [BL, 7, 7] int32

    m = {}
    # xt: [27, NN*BL] with column v*BL + b  (transpose(2,1,0) -> [27, 7, BL])
    xt = np.ascontiguousarray(X.transpose(2, 1, 0).reshape(XD, NN * BL))
    m["xt"] = _bf(xt)
    selfloop = (np.einsum("bvv->bv", adj) > 0).astype(np.float32)  # [BL, 7]
    Xl = X * selfloop[:, :, None]
    m["xloopt"] = _bf(np.ascontiguousarray(Xl.transpose(2, 1, 0).reshape(XD, NN * BL)))
    m["xroott"] = _bf(np.ascontiguousarray(X[:, 0, :X0D].T))

    pm = np.zeros((N_PAIRS, 128, BL), np.float32)
    sm = np.zeros((N_PAIRS, 128, BL), np.float32)
    for x in range(1, NN):
        for v in range(x):
            pid = _pair_id(v, x)
            pm[pid, :, :] = (adj[:, x, v] > 0).astype(np.float32)[None, :]
            sm[pid, :, :] = (adj[:, v, x] > 0).astype(np.float32)[None, :]
    m["pmask"] = _bf(pm)
    m["smask"] = _bf(sm)
    m["pmask8"] = _bf(np.tile(pm, (1, 1, 8)))
    m["smask8"] = _bf(np.tile(sm, (1, 1, 8)))
    return m


def make_in_maps(inputs):
    shared = _prepare_shared(inputs)
    in_maps = []
    for ci in range(N_CORES):
        mm = dict(shared)
        mm.update(_prepare_core(inputs, ci))
        in_maps.append(mm)
    return in_maps


def run_raw(inputs, trace=False, **kw):
    nc, _ = _build()
    in_maps = make_in_maps(inputs)
    res = run_bass_kernel_spmd(nc, in_maps, core_ids=list(range(N_CORES)),
                               trace=trace, **kw)
    mu = np.concatenate([res.results[ci]["mu"] for ci in range(N_CORES)], axis=0)
    std = np.concatenate([res.results[ci]["stdo"] for ci in range(N_CORES)], axis=0)
    return mu.astype(np.float32), std.astype(np.float32), res


def kernel(**inputs):
    mu, std, _ = run_raw(inputs)
    return mu, std


if __name__ == "__main__":
    nc, _ = _build()
    print("build + compile OK")
